# revision 3
# baseline (speedup 1.0000x reference)
"""Trainium2 Bass kernel for nn_LuongAttention.

Reference math (per batch b):
    S   = Dec @ Enc^T          # [T_dec, T_enc]
    Out = S @ Enc              # [T_dec, D]

By associativity:  Out = Dec @ (Enc^T @ Enc) = Dec @ G with G = Enc^T Enc
a [D, D] = [128, 128] Gram matrix.  This removes the [2048, 2048]
intermediate entirely (16x less FLOPs, no big intermediate traffic) and
makes the kernel memory-bound: 3 MiB HBM I/O per core.

Sharding: data-parallel over batch B=8 -> one batch per NeuronCore (8 cores).

Per-core plan (T=2048, D=128, P=128, NT=16 row tiles):
  1. DMA Enc/Dec tiles [128, 128] into SBUF (contiguous 64 KiB loads).
  2. G = sum_i EncTile_i^T @ EncTile_i   (16 accumulating PE matmuls -> PSUM)
  3. For each of 16 Dec tiles:
       DecT_i = PE-transpose(DecTile_i)           (PSUM, via identity)
       copy   -> SBUF
       Out_i  = DecT_i.T @ G = DecTile_i @ G      (PE matmul -> PSUM)
       copy   -> SBUF, DMA to HBM (contiguous 64 KiB store)
"""

import os
import sys
from contextlib import ExitStack

import numpy as np

for _p in (
    "/opt/trn_rl_repo",
    "/root/.axon_site",
    "/root/.axon_site/_ro/trn_rl_repo",
    "/root/.axon_site/_ro/pypackages",
):
    if os.path.isdir(_p) and _p not in sys.path:
        sys.path.append(_p)

import concourse.bacc as bacc
import concourse.mybir as mybir
import concourse.tile as tile
from concourse.bass_utils import run_bass_kernel_spmd
from concourse.masks import make_identity

B, T, D, P = 8, 2048, 128, 128
NT = T // P  # 16 row tiles of 128


def _build_nc():
    nc = bacc.Bacc("TRN2", target_bir_lowering=False, debug=False)
    f32 = mybir.dt.float32

    enc_h = nc.dram_tensor("enc", [T, D], f32, kind="ExternalInput")
    dec_h = nc.dram_tensor("dec", [T, D], f32, kind="ExternalInput")
    out_h = nc.dram_tensor("out", [T, D], f32, kind="ExternalOutput")

    enc_t = enc_h.ap().rearrange("(n p) d -> n p d", p=P)  # [NT, 128, 128]
    dec_t = dec_h.ap().rearrange("(n p) d -> n p d", p=P)
    out_t = out_h.ap().rearrange("(n p) d -> n p d", p=P)

    with ExitStack() as ctx:
        tc = ctx.enter_context(tile.TileContext(nc))
        singles = ctx.enter_context(tc.tile_pool(name="singles", bufs=1))
        work = ctx.enter_context(tc.tile_pool(name="work", bufs=4))
        psum = ctx.enter_context(tc.tile_pool(name="psum", bufs=3, space="PSUM"))
        gpsum = ctx.enter_context(tc.tile_pool(name="gpsum", bufs=1, space="PSUM"))

        ident = singles.tile([P, P], f32)
        make_identity(nc, ident)

        enc_sb = singles.tile([P, NT, D], f32)
        dec_sb = singles.tile([P, NT, D], f32)
        for i in range(NT):
            nc.sync.dma_start(out=enc_sb[:, i, :], in_=enc_t[i])
        for i in range(NT):
            nc.sync.dma_start(out=dec_sb[:, i, :], in_=dec_t[i])

        # G = Enc^T @ Enc accumulated over the 16 row tiles.
        g_ps = gpsum.tile([P, P], f32)
        for i in range(NT):
            nc.tensor.matmul(
                g_ps[:],
                lhsT=enc_sb[:, i, :],
                rhs=enc_sb[:, i, :],
                start=(i == 0),
                stop=(i == NT - 1),
            )
        g_sb = singles.tile([P, P], f32)
        nc.vector.tensor_copy(g_sb[:], g_ps[:])

        for i in range(NT):
            tp = psum.tile([P, P], f32, tag="tp")
            nc.tensor.transpose(out=tp[:], in_=dec_sb[:, i, :], identity=ident[:])
            dect = work.tile([P, P], f32, tag="dect")
            nc.vector.tensor_copy(dect[:], tp[:])

            op = psum.tile([P, P], f32, tag="op")
            nc.tensor.matmul(op[:], lhsT=dect[:], rhs=g_sb[:], start=True, stop=True)
            ot = work.tile([P, P], f32, tag="ot")
            # Alternate PSUM->SBUF copies between ScalarE and VectorE so
            # neither engine serializes the epilogue.
            if i % 2 == 0:
                nc.scalar.copy(ot[:], op[:])
            else:
                nc.vector.tensor_copy(ot[:], op[:])
            nc.sync.dma_start(out=out_t[i], in_=ot[:])

    nc.compile()
    return nc


_NC = None


def _get_nc():
    global _NC
    if _NC is None:
        _NC = _build_nc()
    return _NC


def _run(enc, dec, **kwargs):
    nc = _get_nc()
    in_maps = [{"enc": enc[b], "dec": dec[b]} for b in range(B)]
    res = run_bass_kernel_spmd(nc, in_maps, core_ids=list(range(B)), **kwargs)
    out = np.stack([res.results[b]["out"] for b in range(B)], axis=0)
    return out, res


def kernel(encoder_hidden_states, decoder_hidden_states):
    enc = np.ascontiguousarray(np.asarray(encoder_hidden_states, dtype=np.float32))
    dec = np.ascontiguousarray(np.asarray(decoder_hidden_states, dtype=np.float32))
    assert enc.shape == (B, T, D) and dec.shape == (B, T, D)
    out, _ = _run(enc, dec)
    return out


# revision 4
# speedup vs baseline: 1.2587x; 1.2587x over previous
"""Trainium2 Bass kernel for nn_LuongAttention.

Reference math (per batch b):
    S   = Dec @ Enc^T          # [T_dec, T_enc]
    Out = S @ Enc              # [T_dec, D]

By associativity:  Out = Dec @ (Enc^T @ Enc) = Dec @ G with G = Enc^T Enc
a [D, D] = [128, 128] Gram matrix.  This removes the [2048, 2048]
intermediate entirely (16x less FLOPs, no big intermediate traffic) and
makes the kernel memory-bound: 3 MiB HBM I/O per core.

Sharding: data-parallel over batch B=8 -> one batch per NeuronCore (8 cores).

Per-core plan (T=2048, D=128, P=128, NT=16 row tiles):
  1. DMA Enc/Dec into SBUF in [128, NT, 128] layout (chunked, two HWDGE rings).
  2. G = sum_i EncTile_i^T @ EncTile_i   (16 accumulating PE matmuls -> PSUM)
  3. For each of 16 Dec tiles:
       DecT_i = PE-transpose(DecTile_i)           (PSUM, via identity)
       copy   -> SBUF
       Out_i  = DecT_i.T @ G = DecTile_i @ G      (PE matmul -> PSUM)
       copy   -> SBUF out staging
  4. Chunked DMA stores of the output.
"""

import os
import sys
from contextlib import ExitStack

import numpy as np

for _p in (
    "/opt/trn_rl_repo",
    "/root/.axon_site",
    "/root/.axon_site/_ro/trn_rl_repo",
    "/root/.axon_site/_ro/pypackages",
):
    if os.path.isdir(_p) and _p not in sys.path:
        sys.path.append(_p)

import concourse.bacc as bacc
import concourse.mybir as mybir
import concourse.tile as tile
from concourse.bass_utils import run_bass_kernel_spmd
from concourse.masks import make_identity

B, T, D, P = 8, 2048, 128, 128
NT = T // P  # 16 row tiles of 128

# tunables
ENC_CHUNKS = 4
DEC_CHUNKS = 4
OUT_CHUNKS = 4


def _build_nc():
    nc = bacc.Bacc("TRN2", target_bir_lowering=False, debug=False)
    f32 = mybir.dt.float32

    enc_h = nc.dram_tensor("enc", [T, D], f32, kind="ExternalInput")
    dec_h = nc.dram_tensor("dec", [T, D], f32, kind="ExternalInput")
    out_h = nc.dram_tensor("out", [T, D], f32, kind="ExternalOutput")

    # [p, n, d] views of the HBM tensors (p = row within tile, n = tile)
    enc_v = enc_h.ap().rearrange("(n p) d -> p n d", p=P)
    dec_v = dec_h.ap().rearrange("(n p) d -> p n d", p=P)
    out_v = out_h.ap().rearrange("(n p) d -> p n d", p=P)

    with ExitStack() as ctx:
        tc = ctx.enter_context(tile.TileContext(nc))
        singles = ctx.enter_context(tc.tile_pool(name="singles", bufs=1))
        work = ctx.enter_context(tc.tile_pool(name="work", bufs=4))
        psum = ctx.enter_context(tc.tile_pool(name="psum", bufs=3, space="PSUM"))
        gpsum = ctx.enter_context(tc.tile_pool(name="gpsum", bufs=1, space="PSUM"))

        ident = singles.tile([P, P], f32)
        make_identity(nc, ident)

        enc_sb = singles.tile([P, NT, D], f32)
        dec_sb = singles.tile([P, NT, D], f32)
        out_sb = singles.tile([P, NT, D], f32)

        # Chunked loads: enc on the SP (sync) HWDGE ring, dec on the ACT
        # (scalar) ring so neither sequencer serializes all issues.
        cs = NT // ENC_CHUNKS
        for c in range(ENC_CHUNKS):
            nc.sync.dma_start(
                out=enc_sb[:, c * cs : (c + 1) * cs, :],
                in_=enc_v[:, c * cs : (c + 1) * cs, :],
            )
        cs = NT // DEC_CHUNKS
        for c in range(DEC_CHUNKS):
            nc.scalar.dma_start(
                out=dec_sb[:, c * cs : (c + 1) * cs, :],
                in_=dec_v[:, c * cs : (c + 1) * cs, :],
            )

        # G = Enc^T @ Enc accumulated over the 16 row tiles.
        g_ps = gpsum.tile([P, P], f32)
        for i in range(NT):
            nc.tensor.matmul(
                g_ps[:],
                lhsT=enc_sb[:, i, :],
                rhs=enc_sb[:, i, :],
                start=(i == 0),
                stop=(i == NT - 1),
            )
        g_sb = singles.tile([P, P], f32)
        nc.vector.tensor_copy(g_sb[:], g_ps[:])

        for i in range(NT):
            tp = psum.tile([P, P], f32, tag="tp")
            nc.tensor.transpose(out=tp[:], in_=dec_sb[:, i, :], identity=ident[:])
            dect = work.tile([P, P], f32, tag="dect")
            nc.vector.tensor_copy(dect[:], tp[:])

            op = psum.tile([P, P], f32, tag="op")
            nc.tensor.matmul(op[:], lhsT=dect[:], rhs=g_sb[:], start=True, stop=True)
            nc.vector.tensor_copy(out_sb[:, i, :], op[:])

        cs = NT // OUT_CHUNKS
        for c in range(OUT_CHUNKS):
            nc.sync.dma_start(
                out=out_v[:, c * cs : (c + 1) * cs, :],
                in_=out_sb[:, c * cs : (c + 1) * cs, :],
            )

    nc.compile()
    return nc


_NC = None


def _get_nc():
    global _NC
    if _NC is None:
        _NC = _build_nc()
    return _NC


def _run(enc, dec, **kwargs):
    nc = _get_nc()
    in_maps = [{"enc": enc[b], "dec": dec[b]} for b in range(B)]
    res = run_bass_kernel_spmd(nc, in_maps, core_ids=list(range(B)), **kwargs)
    out = np.stack([res.results[b]["out"] for b in range(B)], axis=0)
    return out, res


def kernel(encoder_hidden_states, decoder_hidden_states):
    enc = np.ascontiguousarray(np.asarray(encoder_hidden_states, dtype=np.float32))
    dec = np.ascontiguousarray(np.asarray(decoder_hidden_states, dtype=np.float32))
    assert enc.shape == (B, T, D) and dec.shape == (B, T, D)
    out, _ = _run(enc, dec)
    return out


# revision 7
# speedup vs baseline: 1.6137x; 1.2820x over previous
"""Trainium2 Bass kernel for nn_LuongAttention.

Reference math (per batch b):
    S   = Dec @ Enc^T          # [T_dec, T_enc]
    Out = S @ Enc              # [T_dec, D]

By associativity:  Out = Dec @ (Enc^T @ Enc) = Dec @ G with G = Enc^T Enc
a [D, D] = [128, 128] Gram matrix.  This removes the [2048, 2048]
intermediate entirely (16x less FLOPs) and makes the kernel
memory-bound: 3 MiB HBM I/O per core.

Sharding: data-parallel over batch B=8 -> one batch per NeuronCore.

Device-side layout trick: the host feeds Dec pre-transposed (DecT
[D, T]) and receives Out transposed (OutT [D, T]); host transposes the
result back during the gather.  With that:
  - G = sum_i EncTile_i^T @ EncTile_i  (accumulating PE matmuls, natural
    encoder layout - no transposes needed)
  - OutT = G @ DecT computed as matmul(lhsT=G, rhs=DecT chunk) with a
    single stationary-weight load and wide moving operand
  - no PE transposes, no identity, minimal PSUM<->SBUF copies
"""

import os
import sys
from contextlib import ExitStack

import numpy as np

for _p in (
    "/opt/trn_rl_repo",
    "/root/.axon_site",
    "/root/.axon_site/_ro/trn_rl_repo",
    "/root/.axon_site/_ro/pypackages",
):
    if os.path.isdir(_p) and _p not in sys.path:
        sys.path.append(_p)

import concourse.bacc as bacc
import concourse.mybir as mybir
import concourse.tile as tile
from concourse.bass_utils import run_bass_kernel_spmd

B, T, D, P = 8, 2048, 128, 128
NT = T // P  # 16 row tiles of 128

# tunables
MM_DTYPE = "f32r"  # "fp32" | "f32r" | "bf16"
ENC_CHUNKS = 4
DEC_CHUNKS = 4
OUT_CHUNKS = 4
FINAL_N = 512  # moving-operand width of the final matmul


def _build_nc(mm_dtype=None):
    mm_dtype = mm_dtype or MM_DTYPE
    nc = bacc.Bacc("TRN2", target_bir_lowering=False, debug=False)
    f32 = mybir.dt.float32
    f32r = mybir.dt.float32r
    bf16 = mybir.dt.bfloat16

    # float32r is 4-byte fp32 storage with reduced-precision PE multiplies;
    # the BIR verifier requires every producer of an f32r matmul input to
    # carry the f32r dtype, so the DRAM/SBUF tensors are declared f32r
    # end-to-end (numpy side stays np.float32).
    in_dt = {"bf16": bf16, "f32r": f32r}.get(mm_dtype, f32)

    enc_h = nc.dram_tensor("enc", [T, D], in_dt, kind="ExternalInput")
    dect_h = nc.dram_tensor("dect", [D, T], in_dt, kind="ExternalInput")
    out_h = nc.dram_tensor("out", [D, T], f32, kind="ExternalOutput")

    # [p, n, d] view of encoder (p = row within tile, n = tile index)
    enc_v = enc_h.ap().rearrange("(n p) d -> p n d", p=P)
    dect_v = dect_h.ap()
    out_v = out_h.ap()

    def mmcast(ap):
        return ap

    with ExitStack() as ctx:
        tc = ctx.enter_context(tile.TileContext(nc))
        singles = ctx.enter_context(tc.tile_pool(name="singles", bufs=1))
        psum = ctx.enter_context(tc.tile_pool(name="psum", bufs=3, space="PSUM"))
        gpsum = ctx.enter_context(tc.tile_pool(name="gpsum", bufs=1, space="PSUM"))

        enc_sb = singles.tile([P, NT, D], in_dt)
        dect_sb = singles.tile([P, T], in_dt)
        out_sb = singles.tile([P, T], f32)

        # Chunked loads: enc on the SP (sync) HWDGE ring, decT on the ACT
        # (scalar) ring so neither sequencer serializes all issues.
        cs = NT // ENC_CHUNKS
        for c in range(ENC_CHUNKS):
            nc.sync.dma_start(
                out=enc_sb[:, c * cs : (c + 1) * cs, :],
                in_=enc_v[:, c * cs : (c + 1) * cs, :],
            )
        cs = T // DEC_CHUNKS
        for c in range(DEC_CHUNKS):
            nc.scalar.dma_start(
                out=dect_sb[:, c * cs : (c + 1) * cs],
                in_=dect_v[:, c * cs : (c + 1) * cs],
            )

        # G = Enc^T @ Enc accumulated over the 16 row tiles.
        g_sb = singles.tile([P, P], in_dt)
        if mm_dtype == "f32r":
            # Pair tiles: rhs spans two adjacent tiles (N=256) so float32r
            # runs at 1 cycle/row instead of 4.  Each mm's useful half:
            #   psum_a left  half = sum_i even-lhsT x own tile
            #   psum_b right half = sum_i odd-lhsT  x own tile
            ps_a = gpsum.tile([P, 2 * P], f32, tag="ga")
            ps_b = gpsum.tile([P, 2 * P], f32, tag="gb")
            for pair in range(NT // 2):
                i, j = 2 * pair, 2 * pair + 1
                rhs = enc_sb[:, i : i + 2, :]
                nc.tensor.matmul(
                    ps_a[:],
                    lhsT=mmcast(enc_sb[:, i, :]),
                    rhs=mmcast(rhs),
                    start=(pair == 0),
                    stop=(pair == NT // 2 - 1),
                )
                nc.tensor.matmul(
                    ps_b[:],
                    lhsT=mmcast(enc_sb[:, j, :]),
                    rhs=mmcast(rhs),
                    start=(pair == 0),
                    stop=(pair == NT // 2 - 1),
                )
            # DVE may read only one PSUM operand per instruction.
            ga_sb = singles.tile([P, P], f32)
            nc.vector.tensor_copy(ga_sb[:], ps_a[:, :P])
            nc.vector.tensor_add(g_sb[:], ga_sb[:], ps_b[:, P:])
        else:
            g_ps = gpsum.tile([P, P], f32, tag="ga")
            for i in range(NT):
                nc.tensor.matmul(
                    g_ps[:],
                    lhsT=enc_sb[:, i, :],
                    rhs=enc_sb[:, i, :],
                    start=(i == 0),
                    stop=(i == NT - 1),
                )
            nc.vector.tensor_copy(g_sb[:], g_ps[:])

        # OutT = G @ DecT  (G symmetric, so lhsT=G computes G.T@X = G@X).
        # One stationary load of G, wide moving chunks of DecT.
        n_final = T // FINAL_N
        for c in range(n_final):
            op = psum.tile([P, FINAL_N], f32, tag="op")
            nc.tensor.matmul(
                op[:],
                lhsT=mmcast(g_sb[:]),
                rhs=mmcast(dect_sb[:, c * FINAL_N : (c + 1) * FINAL_N]),
                start=True,
                stop=True,
            )
            nc.vector.tensor_copy(out_sb[:, c * FINAL_N : (c + 1) * FINAL_N], op[:])

        cs = T // OUT_CHUNKS
        for c in range(OUT_CHUNKS):
            eng = nc.sync if c % 2 == 0 else nc.scalar
            eng.dma_start(
                out=out_v[:, c * cs : (c + 1) * cs],
                in_=out_sb[:, c * cs : (c + 1) * cs],
            )

    nc.compile()
    return nc


_NC = {}


def _get_nc(mm_dtype=None):
    mm_dtype = mm_dtype or MM_DTYPE
    if mm_dtype not in _NC:
        _NC[mm_dtype] = _build_nc(mm_dtype)
    return _NC[mm_dtype]


def _run(enc, dec, mm_dtype=None, **kwargs):
    mm_dtype = mm_dtype or MM_DTYPE
    nc = _get_nc(mm_dtype)
    if mm_dtype == "bf16":
        import ml_dtypes

        np_dt = ml_dtypes.bfloat16
    else:
        np_dt = np.float32
    in_maps = []
    for b in range(B):
        in_maps.append(
            {
                "enc": np.ascontiguousarray(enc[b].astype(np_dt)),
                "dect": np.ascontiguousarray(dec[b].T.astype(np_dt)),
            }
        )
    res = run_bass_kernel_spmd(nc, in_maps, core_ids=list(range(B)), **kwargs)
    out = np.stack([res.results[b]["out"].T for b in range(B)], axis=0)
    return np.ascontiguousarray(out), res


def kernel(encoder_hidden_states, decoder_hidden_states):
    enc = np.ascontiguousarray(np.asarray(encoder_hidden_states, dtype=np.float32))
    dec = np.ascontiguousarray(np.asarray(decoder_hidden_states, dtype=np.float32))
    assert enc.shape == (B, T, D) and dec.shape == (B, T, D)
    out, _ = _run(enc, dec)
    return out


# revision 8
# speedup vs baseline: 1.8513x; 1.1473x over previous
"""Trainium2 Bass kernel for nn_LuongAttention.

Reference math (per batch b):
    S   = Dec @ Enc^T          # [T_dec, T_enc]
    Out = S @ Enc              # [T_dec, D]

By associativity:  Out = Dec @ (Enc^T @ Enc) = Dec @ G with G = Enc^T Enc
a [D, D] = [128, 128] Gram matrix.  This removes the [2048, 2048]
intermediate entirely (16x less FLOPs) and makes the kernel
memory-bound: ~3 MiB HBM I/O per core.

Sharding: data-parallel over batch B=8 -> one batch per NeuronCore.

Device-side layout trick: the host feeds Dec pre-transposed (DecT
[D, T]) and receives Out transposed (OutT [D, T]); the host transposes
the result back during the gather (pure layout permutation, no math).
With that:
  - G = sum_i EncTile_i^T @ EncTile_i  (accumulating PE matmuls, natural
    encoder layout - no transposes needed)
  - OutT = G @ DecT computed as matmul(lhsT=G, rhs=DecT chunk) with wide
    moving chunks (G is symmetric so lhsT=G gives G.T@X = G@X)
  - no PE transposes, no identity, minimal PSUM->SBUF copies
"""

import os
import sys
from contextlib import ExitStack

import numpy as np

for _p in (
    "/opt/trn_rl_repo",
    "/root/.axon_site",
    "/root/.axon_site/_ro/trn_rl_repo",
    "/root/.axon_site/_ro/pypackages",
):
    if os.path.isdir(_p) and _p not in sys.path:
        sys.path.append(_p)

import concourse.bacc as bacc
import concourse.mybir as mybir
import concourse.tile as tile
from concourse.bass_utils import run_bass_kernel_spmd

B, T, D, P = 8, 2048, 128, 128
NT = T // P  # 16 row tiles of 128

# tunables
MM_DTYPE = "f32r"  # "fp32" | "f32r" | "bf16" | "fp16"
ENC_CHUNKS = 8  # tiles per chunk = NT / ENC_CHUNKS
DEC_CHUNKS = 4
FINAL_N = 512  # moving-operand width of the final matmul
COPY_N = 256  # PSUM->SBUF copy / store granularity


def _build_nc(mm_dtype=None):
    mm_dtype = mm_dtype or MM_DTYPE
    nc = bacc.Bacc("TRN2", target_bir_lowering=False, debug=False)
    f32 = mybir.dt.float32
    f32r = mybir.dt.float32r
    bf16 = mybir.dt.bfloat16
    fp16 = mybir.dt.float16

    # float32r is 4-byte fp32 storage with reduced-precision PE multiplies;
    # the BIR verifier requires every producer of an f32r matmul input to
    # carry the f32r dtype, so DRAM/SBUF tensors are declared f32r
    # end-to-end (numpy side stays np.float32).
    in_dt = {"bf16": bf16, "fp16": fp16, "f32r": f32r}.get(mm_dtype, f32)

    enc_h = nc.dram_tensor("enc", [T, D], in_dt, kind="ExternalInput")
    dect_h = nc.dram_tensor("dect", [D, T], in_dt, kind="ExternalInput")
    out_h = nc.dram_tensor("out", [D, T], f32, kind="ExternalOutput")

    # [p, n, d] view of encoder (p = row within tile, n = tile index)
    enc_v = enc_h.ap().rearrange("(n p) d -> p n d", p=P)
    dect_v = dect_h.ap()
    out_v = out_h.ap()

    with ExitStack() as ctx:
        tc = ctx.enter_context(tile.TileContext(nc))
        singles = ctx.enter_context(tc.tile_pool(name="singles", bufs=1))
        psum = ctx.enter_context(tc.tile_pool(name="psum", bufs=3, space="PSUM"))
        gpsum = ctx.enter_context(tc.tile_pool(name="gpsum", bufs=1, space="PSUM"))

        enc_sb = singles.tile([P, NT, D], in_dt)
        dect_sb = singles.tile([P, T], in_dt)
        out_sb = singles.tile([P, T], f32)

        # Interleave chunked loads across both HWDGE rings (SP=sync,
        # ACT=scalar); encoder first on each ring since G consumes it first.
        cs = NT // ENC_CHUNKS
        for c in range(ENC_CHUNKS):
            eng = nc.sync if c % 2 == 0 else nc.scalar
            eng.dma_start(
                out=enc_sb[:, c * cs : (c + 1) * cs, :],
                in_=enc_v[:, c * cs : (c + 1) * cs, :],
            )
        cs = T // DEC_CHUNKS
        for c in range(DEC_CHUNKS):
            eng = nc.sync if c % 2 == 0 else nc.scalar
            eng.dma_start(
                out=dect_sb[:, c * cs : (c + 1) * cs],
                in_=dect_v[:, c * cs : (c + 1) * cs],
            )

        # G = Enc^T @ Enc accumulated over the 16 row tiles.
        g_sb = singles.tile([P, P], in_dt)
        if mm_dtype == "f32r":
            # Pair tiles: rhs spans two adjacent tiles (N=256) so float32r
            # runs at 1 cycle/row instead of 4.  Useful halves:
            #   ps_a left  half accumulates even-tile Grams
            #   ps_b right half accumulates odd-tile Grams
            ps_a = gpsum.tile([P, 2 * P], f32, tag="ga")
            ps_b = gpsum.tile([P, 2 * P], f32, tag="gb")
            for pair in range(NT // 2):
                i, j = 2 * pair, 2 * pair + 1
                rhs = enc_sb[:, i : i + 2, :]
                nc.tensor.matmul(
                    ps_a[:],
                    lhsT=enc_sb[:, i, :],
                    rhs=rhs,
                    start=(pair == 0),
                    stop=(pair == NT // 2 - 1),
                )
                nc.tensor.matmul(
                    ps_b[:],
                    lhsT=enc_sb[:, j, :],
                    rhs=rhs,
                    start=(pair == 0),
                    stop=(pair == NT // 2 - 1),
                )
            # DVE may read only one PSUM operand per instruction.
            ga_sb = singles.tile([P, P], f32)
            nc.vector.tensor_copy(ga_sb[:], ps_a[:, :P])
            nc.vector.tensor_add(g_sb[:], ga_sb[:], ps_b[:, P:])
        else:
            g_ps = gpsum.tile([P, P], f32, tag="ga")
            for i in range(NT):
                nc.tensor.matmul(
                    g_ps[:],
                    lhsT=enc_sb[:, i, :],
                    rhs=enc_sb[:, i, :],
                    start=(i == 0),
                    stop=(i == NT - 1),
                )
            nc.vector.tensor_copy(g_sb[:], g_ps[:])

        # OutT = G @ DecT: wide moving chunks, stationary G.
        # Pipeline: PE matmul -> (DVE|ACT) PSUM->SBUF copy -> chunked store.
        n_final = T // FINAL_N
        ncopy = FINAL_N // COPY_N
        for c in range(n_final):
            op = psum.tile([P, FINAL_N], f32, tag="op")
            nc.tensor.matmul(
                op[:],
                lhsT=g_sb[:],
                rhs=dect_sb[:, c * FINAL_N : (c + 1) * FINAL_N],
                start=True,
                stop=True,
            )
            for k in range(ncopy):
                idx = c * ncopy + k
                lo = c * FINAL_N + k * COPY_N
                eng = nc.vector if idx % 2 == 0 else nc.scalar
                if idx % 2 == 0:
                    nc.vector.tensor_copy(
                        out_sb[:, lo : lo + COPY_N], op[:, k * COPY_N : (k + 1) * COPY_N]
                    )
                else:
                    nc.scalar.copy(
                        out_sb[:, lo : lo + COPY_N], op[:, k * COPY_N : (k + 1) * COPY_N]
                    )
                deng = nc.sync if idx % 2 == 0 else nc.scalar
                deng.dma_start(
                    out=out_v[:, lo : lo + COPY_N],
                    in_=out_sb[:, lo : lo + COPY_N],
                )

    nc.compile()
    return nc


_NC = {}


def _get_nc(mm_dtype=None):
    mm_dtype = mm_dtype or MM_DTYPE
    if mm_dtype not in _NC:
        _NC[mm_dtype] = _build_nc(mm_dtype)
    return _NC[mm_dtype]


def _np_in_dtype(mm_dtype):
    if mm_dtype == "bf16":
        import ml_dtypes

        return ml_dtypes.bfloat16
    if mm_dtype == "fp16":
        return np.float16
    return np.float32


def _run(enc, dec, mm_dtype=None, **kwargs):
    mm_dtype = mm_dtype or MM_DTYPE
    nc = _get_nc(mm_dtype)
    np_dt = _np_in_dtype(mm_dtype)
    in_maps = []
    for b in range(B):
        in_maps.append(
            {
                "enc": np.ascontiguousarray(enc[b].astype(np_dt)),
                "dect": np.ascontiguousarray(dec[b].T.astype(np_dt)),
            }
        )
    res = run_bass_kernel_spmd(nc, in_maps, core_ids=list(range(B)), **kwargs)
    out = np.stack([res.results[b]["out"].T for b in range(B)], axis=0)
    return np.ascontiguousarray(out), res


def kernel(encoder_hidden_states, decoder_hidden_states):
    enc = np.ascontiguousarray(np.asarray(encoder_hidden_states, dtype=np.float32))
    dec = np.ascontiguousarray(np.asarray(decoder_hidden_states, dtype=np.float32))
    assert enc.shape == (B, T, D) and dec.shape == (B, T, D)
    out, _ = _run(enc, dec)
    return out
